# revision 16
# baseline (speedup 1.0000x reference)
"""MGCN (3-layer RGCN-style message passing) on 8 Trainium2 NeuronCores.

Sharding: edges are sharded by destination-node range, aligned with a
node-range sharding of the output (core c owns nodes [c*NS, (c+1)*NS)).
Each core fully aggregates messages for its own nodes, so no all-reduce
is needed; an AllGather replicates the new node features between layers.

Per 128-node block (edges sorted by dst, T edge tiles of 128):
  - per-tile indirect-DMA gather of source features Xg [128e, 128f]
  - one DVE op builds the block's one-hot stack O_n[e,t,m] = (m == slot_et)
  - one DVE op builds O_w[e,t,b,m] = O_n[e,t,m] * att_etb for ALL tiles
  - per tile, one TensorE matmul accumulates Z^T[f,(b,m)] += Xg^T @ O_w[t]
The block's own rows load via a DIRECT DMA from a core-local tensor
(x0self for layer 0, the previous layer's local output otherwise) and a
matmul with identity transposes them for the root term. The epilogue
applies basis/root with 5 accumulating matmuls, adds bias (+ReLU on
layer 2), and writes the block's output rows. PSUM->SBUF copies run on
the Scalar engine to keep DVE free.

Host side does index prep only: sort edges by dst, tile/pad, gather the
tiny att[edge_type] table rows, fold 1/deg into the edge norm, pre-
transpose everything to partition-major layout (contiguous DMAs), and
cast dtypes. All feature FLOPs run on device.
"""

import math

import numpy as np
import ml_dtypes

import concourse.bass as bass
import concourse.tile as tile
from concourse import bacc, mybir
from concourse.bass_utils import run_bass_kernel_spmd

P = 128
NBAS = 4
N_CORES = 8

BF16 = mybir.dt.bfloat16
F32 = mybir.dt.float32
I32 = mybir.dt.int32

_NP_OF = {BF16: ml_dtypes.bfloat16, F32: np.float32}


def build_program(n_cores, nblk, T, D=128, wdt=BF16):
    """Build the SPMD Bass program (same program for every core)."""
    NS = nblk * P
    NP_ = n_cores * NS
    nc = bacc.Bacc(num_devices=n_cores)
    Alu = mybir.AluOpType
    Act = mybir.ActivationFunctionType

    U8 = mybir.dt.uint8
    x0 = nc.declare_dram_parameter("x0", [NP_, D], wdt, isOutput=False)
    x0self = nc.declare_dram_parameter("x0self", [NS, D], wdt, isOutput=False)
    # per-block packed byte blob: [offs i32 | slot f32 | attE bf16] per
    # partition row -> ONE contiguous DMA per block instead of three
    blob1 = nc.declare_dram_parameter("blob1", [nblk, P, 16 * T], U8, isOutput=False)
    blob2 = nc.declare_dram_parameter("blob2", [nblk, P, 16 * T], U8, isOutput=False)
    basis1 = nc.declare_dram_parameter("basis1", [NBAS, D, D], wdt, isOutput=False)
    basis2 = nc.declare_dram_parameter("basis2", [NBAS, D, D], wdt, isOutput=False)
    root1 = nc.declare_dram_parameter("root1", [D, D], wdt, isOutput=False)
    root2 = nc.declare_dram_parameter("root2", [D, D], wdt, isOutput=False)
    biasT1 = nc.declare_dram_parameter("biasT1", [P, D], wdt, isOutput=False)
    biasT2 = nc.declare_dram_parameter("biasT2", [P, D], wdt, isOutput=False)
    iotaT = nc.declare_dram_parameter("iotaT", [P, P], F32, isOutput=False)
    identT = nc.declare_dram_parameter("identT", [P, P], wdt, isOutput=False)
    outp = nc.declare_dram_parameter("out", [NS, D], F32, isOutput=True)

    # (blob, param-set index, relu)
    layers = [(blob1, 0, False), (blob1, 0, True), (blob2, 1, False)]

    with tile.TileContext(nc) as tc:
        with (
            tc.tile_pool(name="const", bufs=1) as cp,
            tc.tile_pool(name="idx", bufs=10) as ixp,
            tc.tile_pool(name="sb", bufs=4) as sb,
            tc.tile_pool(name="owp", bufs=2) as owp,
            tc.tile_pool(name="xgp", bufs=6) as xgp,
            tc.tile_pool(name="pz", bufs=3, space="PSUM") as pz,
            tc.tile_pool(name="pp", bufs=2, space="PSUM") as pp,
            tc.tile_pool(name="dram", bufs=1, space="DRAM") as dp,
        ):
            iota_sb = cp.tile([P, P], F32, tag="iota")
            nc.sync.dma_start(iota_sb[:], iotaT[:])
            ident_sb = cp.tile([P, P], wdt, tag="ident")
            nc.sync.dma_start(ident_sb[:], identT[:])

            basis_sb = []
            root_sb = []
            bias_sb = []
            for i, (b_h, r_h, bi_h) in enumerate(
                ((basis1, root1, biasT1), (basis2, root2, biasT2))
            ):
                bt = cp.tile([P, NBAS, D], wdt, tag=f"basis{i}", name=f"basis_sb{i}")
                nc.sync.dma_start(bt[:], b_h[:].rearrange("b i o -> i b o"))
                basis_sb.append(bt)
                rt = cp.tile([P, D], wdt, tag=f"root{i}", name=f"root_sb{i}")
                nc.sync.dma_start(rt[:], r_h[:])
                root_sb.append(rt)
                bit = cp.tile([P, D], wdt, tag=f"bias{i}", name=f"bias_sb{i}")
                nc.sync.dma_start(bit[:], bi_h[:])
                bias_sb.append(bit)

            x_cur = x0
            x_self = x0self
            for li, (blob, pi, relu) in enumerate(layers):
                last = li == len(layers) - 1
                if not last:
                    xs = dp.tile([NS, D], wdt, tag=f"xs{li}", name=f"xs{li}")
                    xnext = dp.tile(
                        [NP_, D], wdt, tag=f"xn{li}", name=f"xn{li}",
                        addr_space="Shared",
                    )
                for nb in range(nblk):
                    blob_sb = ixp.tile(
                        [P, 16 * T], U8, tag="blob", name=f"blob_{li}_{nb}"
                    )
                    nc.sync.dma_start(blob_sb[:], blob[nb])
                    offs_sb = blob_sb[:, 0 : 4 * T].bitcast(I32)
                    slot_sb = blob_sb[:, 4 * T : 8 * T].bitcast(F32)
                    attE_sb = blob_sb[:, 8 * T : 16 * T].bitcast(wdt).rearrange(
                        "p (t b) -> p t b", b=NBAS
                    )

                    # all T one-hot slot matrices of the block in one DVE op
                    ona = sb.tile([P, T, P], wdt, tag="ona", name=f"ona_{li}_{nb}")
                    nc.vector.tensor_tensor(
                        out=ona[:],
                        in0=iota_sb[:, None, :].to_broadcast([P, T, P]),
                        in1=slot_sb[:, :, None].to_broadcast([P, T, P]),
                        op=Alu.is_equal,
                    )
                    # all T per-tile (basis x one-hot) weights in one DVE op
                    ow = owp.tile([P, T, NBAS, P], wdt, tag="ow", name=f"ow_{li}_{nb}")
                    nc.vector.tensor_tensor(
                        out=ow[:],
                        in0=ona[:, :, None, :].to_broadcast([P, T, NBAS, P]),
                        in1=attE_sb[:, :, :, None].to_broadcast([P, T, NBAS, P]),
                        op=Alu.mult,
                    )

                    zps = pz.tile([P, NBAS, P], F32, tag="z", name=f"z_{li}_{nb}")
                    for t in range(T):
                        xgt = xgp.tile([P, D], wdt, tag="xg", name=f"xg_{li}_{nb}_{t}")
                        nc.gpsimd.indirect_dma_start(
                            out=xgt[:],
                            out_offset=None,
                            in_=x_cur[:, :],
                            in_offset=bass.IndirectOffsetOnAxis(
                                ap=offs_sb[:, t : t + 1], axis=0
                            ),
                        )
                        nc.tensor.matmul(
                            zps[:],
                            lhsT=xgt[:],
                            rhs=ow[:, t, :, :],
                            start=(t == 0),
                            stop=(t == T - 1),
                        )
                    # self rows: direct DMA from the core-local x source
                    xgs = xgp.tile([P, D], wdt, tag="xgs", name=f"xgs_{li}_{nb}")
                    nc.sync.dma_start(xgs[:], x_self[nb * P : (nb + 1) * P, :])
                    sps = pp.tile([P, P], F32, tag="s", name=f"s_{li}_{nb}")
                    nc.tensor.matmul(
                        sps[:], lhsT=xgs[:], rhs=ident_sb[:], start=True, stop=True
                    )

                    # PSUM -> SBUF on the Scalar engine (keeps DVE free)
                    zt = sb.tile([P, NBAS, P], wdt, tag="zt", name=f"zt_{li}_{nb}")
                    nc.scalar.copy(zt[:], zps[:])
                    xt = sb.tile([P, P], wdt, tag="xt", name=f"xt_{li}_{nb}")
                    nc.scalar.copy(xt[:], sps[:])

                    agg = pp.tile([P, P], F32, tag="agg", name=f"agg_{li}_{nb}")
                    for b in range(NBAS):
                        nc.tensor.matmul(
                            agg[:],
                            lhsT=zt[:, b, :],
                            rhs=basis_sb[pi][:, b, :],
                            start=(b == 0),
                            stop=False,
                        )
                    nc.tensor.matmul(
                        agg[:], lhsT=xt[:], rhs=root_sb[pi][:], start=False, stop=True
                    )

                    ob = sb.tile(
                        [P, D],
                        F32 if last else wdt,
                        tag="ob_f" if last else "ob",
                        name=f"ob_{li}_{nb}",
                    )
                    nc.vector.tensor_tensor(
                        out=ob[:], in0=agg[:], in1=bias_sb[pi][:], op=Alu.add
                    )
                    if relu:
                        nc.vector.tensor_scalar(
                            out=ob[:],
                            in0=ob[:],
                            scalar1=0.0,
                            scalar2=None,
                            op0=Alu.max,
                        )
                    dst_rows = outp if last else xs
                    nc.sync.dma_start(dst_rows[nb * P : (nb + 1) * P, :], ob[:])
                if not last:
                    nc.gpsimd.collective_compute(
                        "AllGather",
                        Alu.bypass,
                        replica_groups=[list(range(n_cores))],
                        ins=[xs[:]],
                        outs=[xnext[:]],
                    )
                    x_cur = xnext
                    x_self = xs
    nc.compile()
    return nc


def prepare_inputs(
    entity, edge_index, edge_type, edge_norm, emb,
    att1, att2, basis1, basis2, root1, root2, bias1, bias2,
    n_cores=N_CORES, wdt=BF16,
):
    """Host-side index prep + sharding. Returns (in_maps, nblk, T, N, NS)."""
    npdt = _NP_OF[wdt]
    N = int(entity.shape[0])
    D = int(emb.shape[1])
    x_full = np.asarray(emb, np.float32)[np.asarray(entity, np.int64)]
    src = np.asarray(edge_index[0], np.int64)
    dst = np.asarray(edge_index[1], np.int64)
    et = np.asarray(edge_type, np.int64)
    norm = np.asarray(edge_norm, np.float32)

    NS = ((N + n_cores * P - 1) // (n_cores * P)) * P
    NP_ = NS * n_cores
    nblk = NS // P

    cnt = np.bincount(dst, minlength=NP_).astype(np.float32)
    nw_full = norm / np.maximum(cnt, 1.0)[dst]
    attE1_full = np.asarray(att1, np.float32)[et] * nw_full[:, None]
    attE2_full = np.asarray(att2, np.float32)[et] * nw_full[:, None]

    order = np.argsort(dst, kind="stable")
    gb_bounds = np.searchsorted(dst[order], np.arange(0, NP_ + 1, P))
    ecnt = np.diff(gb_bounds)
    T = max(1, int(math.ceil(ecnt.max() / P)))

    nGB = NP_ // P
    # partition-major layouts: arr[gb, p, t] = value of edge (t*128+p)
    offs_a = np.zeros((nGB, T * P), np.int32)
    slot_a = np.full((nGB, T * P), -1.0, np.float32)
    at1_a = np.zeros((nGB, T * P, NBAS), np.float32)
    at2_a = np.zeros((nGB, T * P, NBAS), np.float32)
    for gb in range(nGB):
        lo, hi = gb_bounds[gb], gb_bounds[gb + 1]
        k = hi - lo
        if k == 0:
            continue
        sel = order[lo:hi]
        offs_a[gb, :k] = src[sel]
        slot_a[gb, :k] = dst[sel] - gb * P
        at1_a[gb, :k] = attE1_full[sel]
        at2_a[gb, :k] = attE2_full[sel]

    def pmajor(a, tail):
        # [nGB, T*P, ...] -> [nGB, P, T, ...]
        return np.ascontiguousarray(
            a.reshape((nGB, T, P) + tail).transpose((0, 2, 1) + tuple(
                3 + i for i in range(len(tail))
            ))
        )

    offs_pm = pmajor(offs_a, ())
    slot_pm = pmajor(slot_a, ())
    at1_pm = pmajor(at1_a, (NBAS,))
    at2_pm = pmajor(at2_a, (NBAS,))

    def pack_blob(offs_c, slot_c, attE_c):
        # [nblk,P,T]i32 + [nblk,P,T]f32 + [nblk,P,T,NBAS]wdt -> [nblk,P,16T]u8
        nb_ = offs_c.shape[0]
        b = np.zeros((nb_, P, 16 * T), np.uint8)
        b[..., 0 : 4 * T] = np.ascontiguousarray(offs_c).view(np.uint8).reshape(
            nb_, P, 4 * T
        )
        b[..., 4 * T : 8 * T] = np.ascontiguousarray(slot_c).view(np.uint8).reshape(
            nb_, P, 4 * T
        )
        b[..., 8 * T : 16 * T] = (
            np.ascontiguousarray(attE_c.astype(npdt))
            .view(np.uint8)
            .reshape(nb_, P, 8 * T)
        )
        return b

    x0 = np.zeros((NP_, D), np.float32)
    x0[:N] = x_full

    iotaT = np.tile(np.arange(P, dtype=np.float32), (P, 1))
    identT = np.eye(P, dtype=np.float32)

    x0_w = x0.astype(npdt)
    common = {
        "basis1": np.asarray(basis1, np.float32).astype(npdt),
        "basis2": np.asarray(basis2, np.float32).astype(npdt),
        "root1": np.asarray(root1, np.float32).astype(npdt),
        "root2": np.asarray(root2, np.float32).astype(npdt),
        "biasT1": np.tile(np.asarray(bias1, np.float32)[None, :], (P, 1)).astype(npdt),
        "biasT2": np.tile(np.asarray(bias2, np.float32)[None, :], (P, 1)).astype(npdt),
        "iotaT": iotaT,
        "identT": identT.astype(npdt),
        "x0": x0_w,
    }

    in_maps = []
    for c in range(n_cores):
        s = slice(c * nblk, (c + 1) * nblk)
        in_maps.append(
            dict(
                common,
                x0self=np.ascontiguousarray(x0_w[c * NS : (c + 1) * NS]),
                blob1=pack_blob(offs_pm[s], slot_pm[s], at1_pm[s]),
                blob2=pack_blob(offs_pm[s], slot_pm[s], at2_pm[s]),
            )
        )
    return in_maps, nblk, T, N, NS


_PROGRAM_CACHE = {}


def run(inputs_dict, n_cores=N_CORES, wdt=BF16, trace=False, trace_kwargs=None):
    """Full pipeline: prep, (cached) build, run, unshard. Returns (out, results)."""
    in_maps, nblk, T, N, NS = prepare_inputs(
        inputs_dict["entity"], inputs_dict["edge_index"], inputs_dict["edge_type"],
        inputs_dict["edge_norm"], inputs_dict["emb"],
        inputs_dict["att1"], inputs_dict["att2"],
        inputs_dict["basis1"], inputs_dict["basis2"],
        inputs_dict["root1"], inputs_dict["root2"],
        inputs_dict["bias1"], inputs_dict["bias2"],
        n_cores=n_cores, wdt=wdt,
    )
    key = (n_cores, nblk, T, wdt)
    if key not in _PROGRAM_CACHE:
        _PROGRAM_CACHE[key] = build_program(n_cores, nblk, T, wdt=wdt)
    nc = _PROGRAM_CACHE[key]
    kwargs = {}
    if trace:
        kwargs["trace"] = True
        if trace_kwargs:
            kwargs.update(trace_kwargs)
    res = run_bass_kernel_spmd(nc, in_maps, list(range(n_cores)), **kwargs)
    out = np.concatenate([res.results[c]["out"] for c in range(n_cores)], axis=0)[:N]
    return np.ascontiguousarray(out, dtype=np.float32), res


def kernel(**inputs):
    out, _ = run(inputs)
    return out


# revision 19
# speedup vs baseline: 1.2120x; 1.2120x over previous
"""MGCN (3-layer RGCN-style message passing) on 8 Trainium2 NeuronCores.

Sharding: edges are sharded by destination-node range, aligned with a
node-range sharding of the output (core c owns nodes [c*NS, (c+1)*NS)).
Each core fully aggregates messages for its own nodes, so no all-reduce
is needed; an AllGather replicates the new node features between layers.

Per 128-node block (edges sorted by dst, T edge tiles of 128):
  - per-tile indirect-DMA gather of source features Xg [128e, 128f]
  - one DVE op builds the block's one-hot stack O_n[e,t,m] = (m == slot_et)
  - one DVE op builds O_w[e,t,b,m] = O_n[e,t,m] * att_etb for ALL tiles
  - per tile, one TensorE matmul accumulates Z^T[f,(b,m)] += Xg^T @ O_w[t]
The block's own rows load via a DIRECT DMA from a core-local tensor
(x0self for layer 0, the previous layer's local output otherwise) and a
matmul with identity transposes them for the root term. The epilogue
applies basis/root with 5 accumulating matmuls, adds bias (+ReLU on
layer 2), and writes the block's output rows. PSUM->SBUF copies run on
the Scalar engine to keep DVE free.

Host side does index prep only: sort edges by dst, tile/pad, gather the
tiny att[edge_type] table rows, fold 1/deg into the edge norm, pre-
transpose everything to partition-major layout (contiguous DMAs), and
cast dtypes. All feature FLOPs run on device.
"""

import math

import numpy as np
import ml_dtypes

import concourse.bass as bass
import concourse.tile as tile
from concourse import bacc, mybir
from concourse.bass_utils import run_bass_kernel_spmd

P = 128
NBAS = 4
N_CORES = 8

BF16 = mybir.dt.bfloat16
F32 = mybir.dt.float32
I32 = mybir.dt.int32

_NP_OF = {BF16: ml_dtypes.bfloat16, F32: np.float32}


def build_program(n_cores, nblk, T, D=128, wdt=BF16):
    """Build the SPMD Bass program (same program for every core)."""
    NS = nblk * P
    NP_ = n_cores * NS
    nc = bacc.Bacc(num_devices=n_cores)
    Alu = mybir.AluOpType
    Act = mybir.ActivationFunctionType

    x0 = nc.declare_dram_parameter("x0", [NP_, D], wdt, isOutput=False)
    x0self = nc.declare_dram_parameter("x0self", [NS, D], wdt, isOutput=False)
    # partition-major host layouts: contiguous per-partition DMA rows
    offs = nc.declare_dram_parameter("offs", [nblk, P, T], I32, isOutput=False)
    slot = nc.declare_dram_parameter("slot", [nblk, P, T], F32, isOutput=False)
    # attE* carry att[edge_type] * edge_norm / deg(dst), pre-folded on host
    attE1 = nc.declare_dram_parameter("attE1", [nblk, P, T, NBAS], wdt, isOutput=False)
    attE2 = nc.declare_dram_parameter("attE2", [nblk, P, T, NBAS], wdt, isOutput=False)
    basis1 = nc.declare_dram_parameter("basis1", [NBAS, D, D], wdt, isOutput=False)
    basis2 = nc.declare_dram_parameter("basis2", [NBAS, D, D], wdt, isOutput=False)
    root1 = nc.declare_dram_parameter("root1", [D, D], wdt, isOutput=False)
    root2 = nc.declare_dram_parameter("root2", [D, D], wdt, isOutput=False)
    biasT1 = nc.declare_dram_parameter("biasT1", [P, D], wdt, isOutput=False)
    biasT2 = nc.declare_dram_parameter("biasT2", [P, D], wdt, isOutput=False)
    iotaT = nc.declare_dram_parameter("iotaT", [P, P], F32, isOutput=False)
    identT = nc.declare_dram_parameter("identT", [P, P], wdt, isOutput=False)
    outp = nc.declare_dram_parameter("out", [NS, D], F32, isOutput=True)

    # (attE, param-set index, relu)
    layers = [(attE1, 0, False), (attE1, 0, True), (attE2, 1, False)]

    with tile.TileContext(nc) as tc:
        with (
            tc.tile_pool(name="const", bufs=1) as cp,
            tc.tile_pool(name="idx", bufs=10) as ixp,
            tc.tile_pool(name="sb", bufs=4) as sb,
            tc.tile_pool(name="owp", bufs=4) as owp,
            tc.tile_pool(name="xgp", bufs=6) as xgp,
            tc.tile_pool(name="pz", bufs=3, space="PSUM") as pz,
            tc.tile_pool(name="pp", bufs=2, space="PSUM") as pp,
            tc.tile_pool(name="dram", bufs=1, space="DRAM") as dp,
        ):
            iota_sb = cp.tile([P, P], F32, tag="iota")
            nc.sync.dma_start(iota_sb[:], iotaT[:])
            ident_sb = cp.tile([P, P], wdt, tag="ident")
            nc.sync.dma_start(ident_sb[:], identT[:])

            basis_sb = []
            root_sb = []
            bias_sb = []
            for i, (b_h, r_h, bi_h) in enumerate(
                ((basis1, root1, biasT1), (basis2, root2, biasT2))
            ):
                bt = cp.tile([P, NBAS, D], wdt, tag=f"basis{i}", name=f"basis_sb{i}")
                nc.sync.dma_start(bt[:], b_h[:].rearrange("b i o -> i b o"))
                basis_sb.append(bt)
                rt = cp.tile([P, D], wdt, tag=f"root{i}", name=f"root_sb{i}")
                nc.sync.dma_start(rt[:], r_h[:])
                root_sb.append(rt)
                bit = cp.tile([P, D], wdt, tag=f"bias{i}", name=f"bias_sb{i}")
                nc.sync.dma_start(bit[:], bi_h[:])
                bias_sb.append(bit)

            x_cur = x0
            x_self = x0self
            for li, (attE, pi, relu) in enumerate(layers):
                last = li == len(layers) - 1
                if not last:
                    xs = dp.tile([NS, D], wdt, tag=f"xs{li}", name=f"xs{li}")
                    xnext = dp.tile(
                        [NP_, D], wdt, tag=f"xn{li}", name=f"xn{li}",
                        addr_space="Shared",
                    )
                for nb in range(nblk):
                    offs_sb = ixp.tile([P, T], I32, tag="offs", name=f"offs_{li}_{nb}")
                    nc.sync.dma_start(offs_sb[:], offs[nb])
                    slot_sb = ixp.tile([P, T], F32, tag="slot", name=f"slot_{li}_{nb}")
                    nc.sync.dma_start(slot_sb[:], slot[nb])
                    attE_sb = ixp.tile(
                        [P, T, NBAS], wdt, tag="attE", name=f"attE_{li}_{nb}"
                    )
                    nc.sync.dma_start(attE_sb[:], attE[nb])

                    # all T one-hot slot matrices of the block in one DVE op
                    ona = sb.tile([P, T, P], wdt, tag="ona", name=f"ona_{li}_{nb}")
                    nc.vector.tensor_tensor(
                        out=ona[:],
                        in0=iota_sb[:, None, :].to_broadcast([P, T, P]),
                        in1=slot_sb[:, :, None].to_broadcast([P, T, P]),
                        op=Alu.is_equal,
                    )
                    zps = pz.tile([P, NBAS, P], F32, tag="z", name=f"z_{li}_{nb}")
                    for t in range(T):
                        xgt = xgp.tile([P, D], wdt, tag="xg", name=f"xg_{li}_{nb}_{t}")
                        nc.gpsimd.indirect_dma_start(
                            out=xgt[:],
                            out_offset=None,
                            in_=x_cur[:, :],
                            in_offset=bass.IndirectOffsetOnAxis(
                                ap=offs_sb[:, t : t + 1], axis=0
                            ),
                        )
                        # per-tile ow: small DVE writes, less SBUF-port
                        # pressure against the Q7's SWDGE descriptor scratch
                        ow = owp.tile(
                            [P, NBAS, P], wdt, tag="ow", name=f"ow_{li}_{nb}_{t}"
                        )
                        nc.vector.tensor_tensor(
                            out=ow[:],
                            in0=ona[:, t, :][:, None, :].to_broadcast([P, NBAS, P]),
                            in1=attE_sb[:, t, :][:, :, None].to_broadcast(
                                [P, NBAS, P]
                            ),
                            op=Alu.mult,
                        )
                        nc.tensor.matmul(
                            zps[:],
                            lhsT=xgt[:],
                            rhs=ow[:],
                            start=(t == 0),
                            stop=(t == T - 1),
                        )
                    # self rows: direct DMA from the core-local x source
                    xgs = xgp.tile([P, D], wdt, tag="xgs", name=f"xgs_{li}_{nb}")
                    nc.sync.dma_start(xgs[:], x_self[nb * P : (nb + 1) * P, :])
                    sps = pp.tile([P, P], F32, tag="s", name=f"s_{li}_{nb}")
                    nc.tensor.matmul(
                        sps[:], lhsT=xgs[:], rhs=ident_sb[:], start=True, stop=True
                    )

                    # PSUM -> SBUF on the Scalar engine (keeps DVE free)
                    zt = sb.tile([P, NBAS, P], wdt, tag="zt", name=f"zt_{li}_{nb}")
                    nc.scalar.copy(zt[:], zps[:])
                    xt = sb.tile([P, P], wdt, tag="xt", name=f"xt_{li}_{nb}")
                    nc.scalar.copy(xt[:], sps[:])

                    agg = pp.tile([P, P], F32, tag="agg", name=f"agg_{li}_{nb}")
                    for b in range(NBAS):
                        nc.tensor.matmul(
                            agg[:],
                            lhsT=zt[:, b, :],
                            rhs=basis_sb[pi][:, b, :],
                            start=(b == 0),
                            stop=False,
                        )
                    nc.tensor.matmul(
                        agg[:], lhsT=xt[:], rhs=root_sb[pi][:], start=False, stop=True
                    )

                    ob = sb.tile(
                        [P, D],
                        F32 if last else wdt,
                        tag="ob_f" if last else "ob",
                        name=f"ob_{li}_{nb}",
                    )
                    nc.vector.tensor_tensor(
                        out=ob[:], in0=agg[:], in1=bias_sb[pi][:], op=Alu.add
                    )
                    if relu:
                        nc.vector.tensor_scalar(
                            out=ob[:],
                            in0=ob[:],
                            scalar1=0.0,
                            scalar2=None,
                            op0=Alu.max,
                        )
                    dst_rows = outp if last else xs
                    nc.sync.dma_start(dst_rows[nb * P : (nb + 1) * P, :], ob[:])
                if not last:
                    nc.gpsimd.collective_compute(
                        "AllGather",
                        Alu.bypass,
                        replica_groups=[list(range(n_cores))],
                        ins=[xs[:]],
                        outs=[xnext[:]],
                    )
                    x_cur = xnext
                    x_self = xs
    nc.compile()
    return nc


def prepare_inputs(
    entity, edge_index, edge_type, edge_norm, emb,
    att1, att2, basis1, basis2, root1, root2, bias1, bias2,
    n_cores=N_CORES, wdt=BF16,
):
    """Host-side index prep + sharding. Returns (in_maps, nblk, T, N, NS)."""
    npdt = _NP_OF[wdt]
    N = int(entity.shape[0])
    D = int(emb.shape[1])
    x_full = np.asarray(emb, np.float32)[np.asarray(entity, np.int64)]
    src = np.asarray(edge_index[0], np.int64)
    dst = np.asarray(edge_index[1], np.int64)
    et = np.asarray(edge_type, np.int64)
    norm = np.asarray(edge_norm, np.float32)

    NS = ((N + n_cores * P - 1) // (n_cores * P)) * P
    NP_ = NS * n_cores
    nblk = NS // P

    cnt = np.bincount(dst, minlength=NP_).astype(np.float32)
    nw_full = norm / np.maximum(cnt, 1.0)[dst]
    attE1_full = np.asarray(att1, np.float32)[et] * nw_full[:, None]
    attE2_full = np.asarray(att2, np.float32)[et] * nw_full[:, None]

    order = np.argsort(dst, kind="stable")
    gb_bounds = np.searchsorted(dst[order], np.arange(0, NP_ + 1, P))
    ecnt = np.diff(gb_bounds)
    T = max(1, int(math.ceil(ecnt.max() / P)))

    nGB = NP_ // P
    # partition-major layouts: arr[gb, p, t] = value of edge (t*128+p)
    offs_a = np.zeros((nGB, T * P), np.int32)
    slot_a = np.full((nGB, T * P), -1.0, np.float32)
    at1_a = np.zeros((nGB, T * P, NBAS), np.float32)
    at2_a = np.zeros((nGB, T * P, NBAS), np.float32)
    for gb in range(nGB):
        lo, hi = gb_bounds[gb], gb_bounds[gb + 1]
        k = hi - lo
        if k == 0:
            continue
        sel = order[lo:hi]
        offs_a[gb, :k] = src[sel]
        slot_a[gb, :k] = dst[sel] - gb * P
        at1_a[gb, :k] = attE1_full[sel]
        at2_a[gb, :k] = attE2_full[sel]

    def pmajor(a, tail):
        # [nGB, T*P, ...] -> [nGB, P, T, ...]
        return np.ascontiguousarray(
            a.reshape((nGB, T, P) + tail).transpose((0, 2, 1) + tuple(
                3 + i for i in range(len(tail))
            ))
        )

    offs_pm = pmajor(offs_a, ())
    slot_pm = pmajor(slot_a, ())
    at1_pm = pmajor(at1_a, (NBAS,))
    at2_pm = pmajor(at2_a, (NBAS,))

    x0 = np.zeros((NP_, D), np.float32)
    x0[:N] = x_full

    iotaT = np.tile(np.arange(P, dtype=np.float32), (P, 1))
    identT = np.eye(P, dtype=np.float32)

    x0_w = x0.astype(npdt)
    common = {
        "basis1": np.asarray(basis1, np.float32).astype(npdt),
        "basis2": np.asarray(basis2, np.float32).astype(npdt),
        "root1": np.asarray(root1, np.float32).astype(npdt),
        "root2": np.asarray(root2, np.float32).astype(npdt),
        "biasT1": np.tile(np.asarray(bias1, np.float32)[None, :], (P, 1)).astype(npdt),
        "biasT2": np.tile(np.asarray(bias2, np.float32)[None, :], (P, 1)).astype(npdt),
        "iotaT": iotaT,
        "identT": identT.astype(npdt),
        "x0": x0_w,
    }

    in_maps = []
    for c in range(n_cores):
        s = slice(c * nblk, (c + 1) * nblk)
        in_maps.append(
            dict(
                common,
                x0self=np.ascontiguousarray(x0_w[c * NS : (c + 1) * NS]),
                offs=offs_pm[s],
                slot=slot_pm[s],
                attE1=at1_pm[s].astype(npdt),
                attE2=at2_pm[s].astype(npdt),
            )
        )
    return in_maps, nblk, T, N, NS


_PROGRAM_CACHE = {}


def run(inputs_dict, n_cores=N_CORES, wdt=BF16, trace=False, trace_kwargs=None):
    """Full pipeline: prep, (cached) build, run, unshard. Returns (out, results)."""
    in_maps, nblk, T, N, NS = prepare_inputs(
        inputs_dict["entity"], inputs_dict["edge_index"], inputs_dict["edge_type"],
        inputs_dict["edge_norm"], inputs_dict["emb"],
        inputs_dict["att1"], inputs_dict["att2"],
        inputs_dict["basis1"], inputs_dict["basis2"],
        inputs_dict["root1"], inputs_dict["root2"],
        inputs_dict["bias1"], inputs_dict["bias2"],
        n_cores=n_cores, wdt=wdt,
    )
    key = (n_cores, nblk, T, wdt)
    if key not in _PROGRAM_CACHE:
        _PROGRAM_CACHE[key] = build_program(n_cores, nblk, T, wdt=wdt)
    nc = _PROGRAM_CACHE[key]
    kwargs = {}
    if trace:
        kwargs["trace"] = True
        if trace_kwargs:
            kwargs.update(trace_kwargs)
    res = run_bass_kernel_spmd(nc, in_maps, list(range(n_cores)), **kwargs)
    out = np.concatenate([res.results[c]["out"] for c in range(n_cores)], axis=0)[:N]
    return np.ascontiguousarray(out, dtype=np.float32), res


def kernel(**inputs):
    out, _ = run(inputs)
    return out
